# revision 17
# baseline (speedup 1.0000x reference)
"""KalmanNet SLAM DNN forward pass on a single Trainium2 NeuronCore.

Network: x(14) -> Linear(560)+ReLU -> GRUCell(145) -> GRUCell(145)
         -> Linear(40)+ReLU -> Linear(10) -> reshape (5,2)

~1.8MB of fp32 weights, single sample => memory-bound; replicate on one
core (per sharding hint).

Matvecs run weights-stationary on the TensorEngine in fp32r (single-pass
fp32; ~1e-4-class relative error, far inside the scale-relative gate).
fp32r requires an even moving free dim, so every activation vector is
kept in duplicated column pairs ([K,2] rhs -> [M,2] psum) end to end.

Host-side numpy packs everything into three partition-major DRAM images
(per-partition contiguous runs of 2-14KB => near-line-rate SWDGE
descriptors, ~5% padding total), weights pre-transposed to [K, M]
layout, biases folded as an extra weight row against a constant-1.0
input element, GRU gates padded 145->146 so output chunks are uniform
73 partitions, and the z-gate pad-column bias set to 100 so the h'
garbage slot computes to exactly the 1.0 the next bias row needs.

Pointwise GRU math on VectorE, Sigmoid/Tanh on ScalarE (table-set load
pulled to t=0 by a dummy op), plus a dummy-matmul burst to warm the PE
clock during the DMA window.
"""

import numpy as np

import concourse.bacc as bacc
import concourse.mybir as mybir
import concourse.tile as tile
from concourse import bass_utils

F32 = mybir.dt.float32
F32R = mybir.dt.float32r
AF = mybir.ActivationFunctionType

X_DIM, Y_DIM = 5, 2
H1, H2 = 560, 40
G = 145          # GRU hidden size
C = 73           # partition chunk for the GRU state (2*73 = 146 = G+1)
GP = 2 * C       # per-gate padded column block
M3 = 3 * GP      # 438 padded gate columns
NO = X_DIM * Y_DIM

# megaB (73-partition image) column map
B_H0, B_H1 = 0, 4                      # h pairs [73,4]: (c0,c0,c1,c1)
B_WHH0 = 8                             # 2 x 438
B_WIH0C4 = B_WHH0 + 2 * M3             # [49,438]
B_WIH1 = B_WIH0C4 + M3                 # 2 x 438
B_WHH1 = B_WIH1 + 2 * M3               # 2 x 438
B_W2A = B_WHH1 + 2 * M3                # 2 x 41 (41st col makes the 1.0)
B_W2B = B_W2A + 2 * (H2 + 1)           # [41,10]
B_F = B_W2B + NO                       # 3164

A_F = 4 * M3                           # mega128: wih0 chunks c0..c3
C_F = H1 + 3                           # megaC: W1T(561, unit col) + x pair

TRACE = False
_BUILT = None


def _emit_gru(nc, pp, ab, name, wih_chunks, whh_chunks, h_sb,
              ptag_rz, ptag_ni, ptag_nh):
    """One GRU cell, everything in duplicated column pairs.
    *_chunks: (lhsT[K, 438], rhs[K, 2]); h_sb: [73,4] prev hidden pairs
    with slots (72, 2:4) = 1.0. Returns h' [73,4] F32R pairs."""
    ps_rz = pp.tile([C, 8], F32, tag=ptag_rz)   # (r_c0, r_c1, z_c0, z_c1) pairs
    ps_ni = pp.tile([C, 4], F32, tag=ptag_ni)
    ps_nh = pp.tile([C, 4], F32, tag=ptag_nh)

    nwi, nwh = len(wih_chunks), len(whh_chunks)
    rz_n = 4 * (nwh + nwi)
    rz_i = ni_i = nh_i = 0
    # gi first (its weights arrive first), gh accumulates on top
    for kc, (lhsT, rhs) in enumerate(wih_chunks):
        for g in (0, 1):
            for c in (0, 1):
                j = 2 * g + c
                nc.tensor.matmul(
                    ps_rz[:, 2 * j: 2 * j + 2],
                    lhsT[:, g * GP + C * c: g * GP + C * (c + 1)],
                    rhs, start=(rz_i == 0), stop=(rz_i == rz_n - 1))
                rz_i += 1
        for c in (0, 1):
            nc.tensor.matmul(
                ps_ni[:, 2 * c: 2 * c + 2],
                lhsT[:, 2 * GP + C * c: 2 * GP + C * (c + 1)],
                rhs, start=(ni_i == 0), stop=(ni_i == 2 * nwi - 1))
            ni_i += 1
    for kc, (lhsT, rhs) in enumerate(whh_chunks):
        for g in (0, 1):
            for c in (0, 1):
                j = 2 * g + c
                nc.tensor.matmul(
                    ps_rz[:, 2 * j: 2 * j + 2],
                    lhsT[:, g * GP + C * c: g * GP + C * (c + 1)],
                    rhs, start=(rz_i == 0), stop=(rz_i == rz_n - 1))
                rz_i += 1
        for c in (0, 1):
            nc.tensor.matmul(
                ps_nh[:, 2 * c: 2 * c + 2],
                lhsT[:, 2 * GP + C * c: 2 * GP + C * (c + 1)],
                rhs, start=(nh_i == 0), stop=(nh_i == 2 * nwh - 1))
            nh_i += 1

    # r,z = sigmoid(rz sums); n = tanh(i_n + r*h_n); h' = n + z*(h-n)
    rz = ab.tile([C, 8], F32, tag=f"{name}_rz")
    nc.scalar.activation(rz, ps_rz, AF.Sigmoid)
    t1 = ab.tile([C, 4], F32, tag=f"{name}_t1")
    nc.vector.tensor_mul(t1, rz[:, 0:4], ps_nh)
    nc.vector.tensor_add(t1, t1, ps_ni)
    n_sb = ab.tile([C, 4], F32, tag=f"{name}_n")
    nc.scalar.activation(n_sb, t1, AF.Tanh)
    d = ab.tile([C, 4], F32, tag=f"{name}_d")
    nc.vector.tensor_sub(d, h_sb.bitcast(F32), n_sb)
    nc.vector.tensor_mul(d, d, rz[:, 4:8])
    hp = ab.tile([C, 4], F32R, tag=f"{name}_hp")
    nc.vector.tensor_add(hp, n_sb, d)
    return hp


def _build():
    nc = bacc.Bacc("TRN2", num_devices=1, num_swdge_queues=4)

    d_a = nc.dram_tensor("mega_a", [128, A_F], F32R, kind="ExternalInput").ap()
    d_b = nc.dram_tensor("mega_b", [C, B_F], F32R, kind="ExternalInput").ap()
    d_c = nc.dram_tensor("mega_c", [15, C_F], F32R, kind="ExternalInput").ap()
    d_out = nc.dram_tensor("out", [1, NO], F32, kind="ExternalOutput").ap()

    with tile.TileContext(nc) as tc:
        with (
            tc.tile_pool(name="wp", bufs=1) as wp,
            tc.tile_pool(name="ab", bufs=1) as ab,
            tc.tile_pool(name="pp", bufs=1, space="PSUM") as pp,
        ):
            # ACT table warmup
            warm = ab.tile([1, 1], F32, tag="warm")
            nc.vector.memset(warm, 0.0)
            warm2 = ab.tile([1, 1], F32, tag="warm2")
            nc.scalar.activation(warm2, warm, AF.Sigmoid)
            nc.scalar.activation(warm2, warm2, AF.Tanh)

            # --- DMAs (SWDGE), priority order ---
            mc = wp.tile([15, C_F], F32R, tag="mc")
            nc.sync.dma_start(mc, d_c)        # tiny; separate HWDGE ring
            mb = wp.tile([C, B_F], F32R, tag="mb")
            nc.gpsimd.dma_start(mb[:, 0:B_WIH1], d_b[:, 0:B_WIH1])
            nc.gpsimd.dma_start(mb[:, B_WIH1:B_F], d_b[:, B_WIH1:B_F])
            ma = wp.tile([128, A_F], F32R, tag="ma")
            nc.gpsimd.dma_start(ma, d_a)

            # --- PE warmup: dummy fp32 matmuls (~3.5us of HAM activity) ---
            wz = ab.tile([128, 128], F32, tag="wz")
            nc.vector.memset(wz, 0.0)
            ps_w = pp.tile([2, 128], F32, tag="pw")
            for i in range(8):
                nc.tensor.matmul(ps_w, wz[:, 0:2], wz,
                                 start=(i == 0), stop=(i == 7))

            # --- layer 1: l1 = relu(W1 @ x + b1), [128,10] paired cols ---
            x2 = mc[:, H1 + 1:H1 + 3]
            ps_l1 = pp.tile([128, 8], F32, tag="p0")
            for c in range(4):
                nc.tensor.matmul(ps_l1[:, 2 * c:2 * c + 2],
                                 mc[:, c * 128:(c + 1) * 128], x2,
                                 start=(c == 0), stop=(c == 3))
            ps_l1b = pp.tile([49, 2], F32, tag="p5")
            nc.tensor.matmul(ps_l1b, mc[:, 512:561], x2, start=True, stop=True)
            l1_sb = ab.tile([128, 10], F32R, tag="l1")
            nc.vector.tensor_scalar_max(l1_sb[:, 0:8], ps_l1, 0.0)
            nc.vector.tensor_scalar_max(l1_sb[0:49, 8:10], ps_l1b, 0.0)

            h0_sb = mb[:, B_H0:B_H0 + 4]
            h1_sb = mb[:, B_H1:B_H1 + 4]

            # --- GRU 0 ---
            wih0_chunks = [
                (ma[:, c * M3:(c + 1) * M3], l1_sb[:, 2 * c:2 * c + 2])
                for c in range(4)
            ] + [
                (mb[0:49, B_WIH0C4:B_WIH0C4 + M3], l1_sb[0:49, 8:10])
            ]
            whh0_chunks = [
                (mb[:, B_WHH0 + c * M3: B_WHH0 + (c + 1) * M3],
                 h0_sb[:, 2 * c:2 * c + 2])
                for c in range(2)
            ]
            hp0 = _emit_gru(nc, pp, ab, "g0", wih0_chunks, whh0_chunks, h0_sb,
                            "p1", "p2", "p3")

            # --- GRU 1 ---
            wih1_chunks = [
                (mb[:, B_WIH1 + c * M3: B_WIH1 + (c + 1) * M3],
                 hp0[:, 2 * c:2 * c + 2])
                for c in range(2)
            ]
            whh1_chunks = [
                (mb[:, B_WHH1 + c * M3: B_WHH1 + (c + 1) * M3],
                 h1_sb[:, 2 * c:2 * c + 2])
                for c in range(2)
            ]
            hp1 = _emit_gru(nc, pp, ab, "g1", wih1_chunks, whh1_chunks, h1_sb,
                            "p0", "p1", "p2")

            # --- l2 ---
            ps_a = pp.tile([H2 + 1, 2], F32, tag="p3")
            for c in range(2):
                nc.tensor.matmul(
                    ps_a, mb[:, B_W2A + c * (H2 + 1): B_W2A + (c + 1) * (H2 + 1)],
                    hp1[:, 2 * c:2 * c + 2], start=(c == 0), stop=(c == 1))
            l2h = ab.tile([H2 + 1, 2], F32R, tag="l2h")
            nc.vector.tensor_scalar_max(l2h, ps_a, 0.0)
            ps_o = pp.tile([1, NO], F32, tag="p4")
            nc.tensor.matmul(ps_o, l2h[:, 0:1],
                             mb[0:H2 + 1, B_W2B:B_W2B + NO],
                             start=True, stop=True)
            out_sb = ab.tile([1, NO], F32, tag="out_sb")
            nc.vector.tensor_copy(out_sb, ps_o)
            nc.sync.dma_start(d_out, out_sb)

    nc.compile()
    return nc


def _get_nc():
    global _BUILT
    if _BUILT is None:
        _BUILT = _build()
    return _BUILT


def _gate_pack(W, b, z_pad_bias=0.0):
    """W:(435,K), b:(435,) -> (K+1, 438): W.T + bias row, per-gate 146-col
    blocks (zero pad col). z_pad_bias=100 on the ih matrix makes the h'
    garbage slot compute to exactly 1.0."""
    K = W.shape[1]
    full = np.concatenate([W.T, b[None, :]], axis=0).astype(np.float32)
    out = np.zeros((K + 1, M3), np.float32)
    for g in range(3):
        out[:, g * GP: g * GP + G] = full[:, g * G: (g + 1) * G]
    out[K, GP + G] = z_pad_bias
    return out


def pack_inputs(inputs):
    f = lambda a: np.asarray(a, np.float32)
    wih0 = _gate_pack(f(inputs["Wih0"]), f(inputs["bih0"]), 100.0)  # (561, 438)
    ma = np.zeros((128, A_F), np.float32)
    for c in range(4):
        ma[:, c * M3:(c + 1) * M3] = wih0[c * 128:(c + 1) * 128, :]

    mb = np.zeros((C, B_F), np.float32)
    hn = f(inputs["hn"])
    for col, h in ((B_H0, hn[0]), (B_H1, hn[1])):
        v = np.append(h, np.float32(1.0)).reshape(2, C).T  # [73,2]
        mb[:, col:col + 4] = v[:, [0, 0, 1, 1]]            # paired
    mb[0:49, B_WIH0C4:B_WIH0C4 + M3] = wih0[512:561, :]
    for col, W, b, zb in (
        (B_WHH0, inputs["Whh0"], inputs["bhh0"], 0.0),
        (B_WIH1, inputs["Wih1"], inputs["bih1"], 100.0),
        (B_WHH1, inputs["Whh1"], inputs["bhh1"], 0.0),
    ):
        wt = _gate_pack(f(W), f(b), zb)                    # (146, 438)
        mb[:, col:col + M3] = wt[0:C, :]
        mb[:, col + M3:col + 2 * M3] = wt[C:2 * C, :]
    w2a = np.zeros((2 * C, H2 + 1), np.float32)
    w2a[0:G + 1, 0:H2] = np.concatenate(
        [f(inputs["W2a"]).T, f(inputs["b2a"])[None, :]], axis=0)
    w2a[G, H2] = 1.0                 # unit col -> l2h slot computes to 1.0
    mb[:, B_W2A:B_W2A + H2 + 1] = w2a[0:C, :]
    mb[:, B_W2A + H2 + 1:B_W2A + 2 * (H2 + 1)] = w2a[C:2 * C, :]
    w2b = np.concatenate([f(inputs["W2b"]).T, f(inputs["b2b"])[None, :]], axis=0)
    mb[0:H2 + 1, B_W2B:B_W2B + NO] = w2b

    mc = np.zeros((15, C_F), np.float32)
    mc[:, 0:H1] = np.concatenate(
        [f(inputs["W1"]).T, f(inputs["b1"])[None, :]], axis=0)
    mc[14, H1] = 1.0                 # unit col -> l1 slot computes to 1.0
    x_ext = np.concatenate([
        f(inputs["state_inno"]), f(inputs["obs_inno"]),
        f(inputs["diff_state"]), f(inputs["diff_obs"]), [np.float32(1.0)],
    ])
    mc[:, H1 + 1] = x_ext
    mc[:, H1 + 2] = x_ext
    return {"mega_a": ma, "mega_b": mb, "mega_c": mc}


def kernel(**inputs):
    nc = _get_nc()
    in_map = pack_inputs(inputs)
    res = bass_utils.run_bass_kernel_spmd(nc, [in_map], core_ids=[0], trace=TRACE)
    kernel.last_result = res
    return np.asarray(res.results[0]["out"], np.float32).reshape(X_DIM, Y_DIM)


# revision 18
# speedup vs baseline: 1.1838x; 1.1838x over previous
"""KalmanNet SLAM DNN forward pass on a single Trainium2 NeuronCore.

Network: x(14) -> Linear(560)+ReLU -> GRUCell(145) -> GRUCell(145)
         -> Linear(40)+ReLU -> Linear(10) -> reshape (5,2)

~1.8MB of fp32 weights, single sample => memory-bound; replicate on one
core (per sharding hint).

Matvecs run weights-stationary on the TensorEngine in fp32r (single-pass
fp32; ~1e-4-class relative error, far inside the scale-relative gate).
fp32r requires an even moving free dim, so every activation vector is
kept in duplicated column pairs ([K,2] rhs -> [M,2] psum) end to end.

Host-side numpy packs everything into three partition-major DRAM images
(per-partition contiguous runs of 2-14KB => near-line-rate SWDGE
descriptors, ~5% padding total), weights pre-transposed to [K, M]
layout, biases folded as an extra weight row against a constant-1.0
input element, GRU gates padded 145->146 so output chunks are uniform
73 partitions, and the z-gate pad-column bias set to 100 so the h'
garbage slot computes to exactly the 1.0 the next bias row needs.

Pointwise GRU math on VectorE, Sigmoid/Tanh on ScalarE (table-set load
pulled to t=0 by a dummy op), plus a dummy-matmul burst to warm the PE
clock during the DMA window.
"""

import numpy as np

import concourse.bacc as bacc
import concourse.mybir as mybir
import concourse.tile as tile
from concourse import bass_utils

F32 = mybir.dt.float32
F32R = mybir.dt.float32r
AF = mybir.ActivationFunctionType

X_DIM, Y_DIM = 5, 2
H1, H2 = 560, 40
G = 145          # GRU hidden size
C = 73           # partition chunk for the GRU state (2*73 = 146 = G+1)
GP = 2 * C       # per-gate padded column block
M3 = 3 * GP      # 438 padded gate columns
NO = X_DIM * Y_DIM

# megaB (73-partition image) column map
B_H0, B_H1 = 0, 4                      # h pairs [73,4]: (c0,c0,c1,c1)
B_WHH0 = 8                             # 2 x 438
B_WIH0C4 = B_WHH0 + 2 * M3             # [49,438]
B_WIH1 = B_WIH0C4 + M3                 # 2 x 438
B_WHH1 = B_WIH1 + 2 * M3               # 2 x 438
B_W2A = B_WHH1 + 2 * M3                # 2 x 41 (41st col makes the 1.0)
B_W2B = B_W2A + 2 * (H2 + 1)           # [41,10]
B_F = B_W2B + NO                       # 3164

A_F = 4 * M3                           # mega128: wih0 chunks c0..c3
C_F = H1 + 3                           # megaC: W1T(561, unit col) + x pair

TRACE = False
_BUILT = None


def _emit_gru(nc, pp, ab, name, wih_chunks, whh_chunks, h_sb,
              ptag_rz, ptag_ni, ptag_nh):
    """One GRU cell, everything in duplicated column pairs.
    *_chunks: (lhsT[K, 438], rhs[K, 2]); h_sb: [73,4] prev hidden pairs
    with slots (72, 2:4) = 1.0. Returns h' [73,4] F32R pairs."""
    ps_rz = pp.tile([C, 8], F32, tag=ptag_rz)   # (r_c0, r_c1, z_c0, z_c1) pairs
    ps_ni = pp.tile([C, 4], F32, tag=ptag_ni)
    ps_nh = pp.tile([C, 4], F32, tag=ptag_nh)

    nwi, nwh = len(wih_chunks), len(whh_chunks)
    rz_n = 4 * (nwh + nwi)
    rz_i = ni_i = nh_i = 0
    # gi first (its weights arrive first), gh accumulates on top
    for kc, (lhsT, rhs) in enumerate(wih_chunks):
        for g in (0, 1):
            for c in (0, 1):
                j = 2 * g + c
                nc.tensor.matmul(
                    ps_rz[:, 2 * j: 2 * j + 2],
                    lhsT[:, g * GP + C * c: g * GP + C * (c + 1)],
                    rhs, start=(rz_i == 0), stop=(rz_i == rz_n - 1))
                rz_i += 1
        for c in (0, 1):
            nc.tensor.matmul(
                ps_ni[:, 2 * c: 2 * c + 2],
                lhsT[:, 2 * GP + C * c: 2 * GP + C * (c + 1)],
                rhs, start=(ni_i == 0), stop=(ni_i == 2 * nwi - 1))
            ni_i += 1
    for kc, (lhsT, rhs) in enumerate(whh_chunks):
        for g in (0, 1):
            for c in (0, 1):
                j = 2 * g + c
                nc.tensor.matmul(
                    ps_rz[:, 2 * j: 2 * j + 2],
                    lhsT[:, g * GP + C * c: g * GP + C * (c + 1)],
                    rhs, start=(rz_i == 0), stop=(rz_i == rz_n - 1))
                rz_i += 1
        for c in (0, 1):
            nc.tensor.matmul(
                ps_nh[:, 2 * c: 2 * c + 2],
                lhsT[:, 2 * GP + C * c: 2 * GP + C * (c + 1)],
                rhs, start=(nh_i == 0), stop=(nh_i == 2 * nwh - 1))
            nh_i += 1

    # r,z = sigmoid(rz sums); n = tanh(i_n + r*h_n); h' = n + z*(h-n)
    rz = ab.tile([C, 8], F32, tag=f"{name}_rz")
    nc.scalar.activation(rz, ps_rz, AF.Sigmoid)
    t1 = ab.tile([C, 4], F32, tag=f"{name}_t1")
    nc.vector.tensor_mul(t1, rz[:, 0:4], ps_nh)
    nc.vector.tensor_add(t1, t1, ps_ni)
    n_sb = ab.tile([C, 4], F32, tag=f"{name}_n")
    nc.scalar.activation(n_sb, t1, AF.Tanh)
    d = ab.tile([C, 4], F32, tag=f"{name}_d")
    nc.vector.tensor_sub(d, h_sb.bitcast(F32), n_sb)
    nc.vector.tensor_mul(d, d, rz[:, 4:8])
    hp = ab.tile([C, 4], F32R, tag=f"{name}_hp")
    nc.vector.tensor_add(hp, n_sb, d)
    return hp


def _build():
    nc = bacc.Bacc("TRN2", num_devices=1, num_swdge_queues=4)

    d_a = nc.dram_tensor("mega_a", [128, A_F], F32R, kind="ExternalInput").ap()
    d_b1 = nc.dram_tensor("mega_b1", [C, B_WIH1], F32R, kind="ExternalInput").ap()
    d_b2 = nc.dram_tensor("mega_b2", [C, B_F - B_WIH1], F32R, kind="ExternalInput").ap()
    d_c = nc.dram_tensor("mega_c", [15, C_F], F32R, kind="ExternalInput").ap()
    d_out = nc.dram_tensor("out", [1, NO], F32, kind="ExternalOutput").ap()

    with tile.TileContext(nc) as tc:
        with (
            tc.tile_pool(name="wp", bufs=1) as wp,
            tc.tile_pool(name="ab", bufs=1) as ab,
            tc.tile_pool(name="pp", bufs=1, space="PSUM") as pp,
        ):
            # ACT table warmup
            warm = ab.tile([1, 1], F32, tag="warm")
            nc.vector.memset(warm, 0.0)
            warm2 = ab.tile([1, 1], F32, tag="warm2")
            nc.scalar.activation(warm2, warm, AF.Sigmoid)
            nc.scalar.activation(warm2, warm2, AF.Tanh)

            # --- DMAs (SWDGE), priority order ---
            mc = wp.tile([15, C_F], F32R, tag="mc")
            nc.sync.dma_start(mc, d_c)        # tiny; separate HWDGE ring
            ma = wp.tile([128, A_F], F32R, tag="ma")
            nc.gpsimd.dma_start(ma, d_a)
            mb = wp.tile([C, B_F], F32R, tag="mb")
            nc.gpsimd.dma_start(mb[:, 0:B_WIH1], d_b1)
            nc.gpsimd.dma_start(mb[:, B_WIH1:B_F], d_b2)

            # --- PE warmup: dummy fp32 matmuls (~3.5us of HAM activity) ---
            wz = ab.tile([128, 128], F32, tag="wz")
            nc.vector.memset(wz, 0.0)
            ps_w = pp.tile([2, 128], F32, tag="pw")
            for i in range(8):
                nc.tensor.matmul(ps_w, wz[:, 0:2], wz,
                                 start=(i == 0), stop=(i == 7))

            # --- layer 1: l1 = relu(W1 @ x + b1), [128,10] paired cols ---
            x2 = mc[:, H1 + 1:H1 + 3]
            ps_l1 = pp.tile([128, 8], F32, tag="p0")
            for c in range(4):
                nc.tensor.matmul(ps_l1[:, 2 * c:2 * c + 2],
                                 mc[:, c * 128:(c + 1) * 128], x2,
                                 start=(c == 0), stop=(c == 3))
            ps_l1b = pp.tile([49, 2], F32, tag="p5")
            nc.tensor.matmul(ps_l1b, mc[:, 512:561], x2, start=True, stop=True)
            l1_sb = ab.tile([128, 10], F32R, tag="l1")
            nc.vector.tensor_scalar_max(l1_sb[:, 0:8], ps_l1, 0.0)
            nc.vector.tensor_scalar_max(l1_sb[0:49, 8:10], ps_l1b, 0.0)

            h0_sb = mb[:, B_H0:B_H0 + 4]
            h1_sb = mb[:, B_H1:B_H1 + 4]

            # --- GRU 0 ---
            wih0_chunks = [
                (ma[:, c * M3:(c + 1) * M3], l1_sb[:, 2 * c:2 * c + 2])
                for c in range(4)
            ] + [
                (mb[0:49, B_WIH0C4:B_WIH0C4 + M3], l1_sb[0:49, 8:10])
            ]
            whh0_chunks = [
                (mb[:, B_WHH0 + c * M3: B_WHH0 + (c + 1) * M3],
                 h0_sb[:, 2 * c:2 * c + 2])
                for c in range(2)
            ]
            hp0 = _emit_gru(nc, pp, ab, "g0", wih0_chunks, whh0_chunks, h0_sb,
                            "p1", "p2", "p3")

            # --- GRU 1 ---
            wih1_chunks = [
                (mb[:, B_WIH1 + c * M3: B_WIH1 + (c + 1) * M3],
                 hp0[:, 2 * c:2 * c + 2])
                for c in range(2)
            ]
            whh1_chunks = [
                (mb[:, B_WHH1 + c * M3: B_WHH1 + (c + 1) * M3],
                 h1_sb[:, 2 * c:2 * c + 2])
                for c in range(2)
            ]
            hp1 = _emit_gru(nc, pp, ab, "g1", wih1_chunks, whh1_chunks, h1_sb,
                            "p0", "p1", "p2")

            # --- l2 ---
            ps_a = pp.tile([H2 + 1, 2], F32, tag="p3")
            for c in range(2):
                nc.tensor.matmul(
                    ps_a, mb[:, B_W2A + c * (H2 + 1): B_W2A + (c + 1) * (H2 + 1)],
                    hp1[:, 2 * c:2 * c + 2], start=(c == 0), stop=(c == 1))
            l2h = ab.tile([H2 + 1, 2], F32R, tag="l2h")
            nc.vector.tensor_scalar_max(l2h, ps_a, 0.0)
            ps_o = pp.tile([1, NO], F32, tag="p4")
            nc.tensor.matmul(ps_o, l2h[:, 0:1],
                             mb[0:H2 + 1, B_W2B:B_W2B + NO],
                             start=True, stop=True)
            out_sb = ab.tile([1, NO], F32, tag="out_sb")
            nc.vector.tensor_copy(out_sb, ps_o)
            nc.sync.dma_start(d_out, out_sb)

    nc.compile()
    return nc


def _get_nc():
    global _BUILT
    if _BUILT is None:
        _BUILT = _build()
    return _BUILT


def _gate_pack(W, b, z_pad_bias=0.0):
    """W:(435,K), b:(435,) -> (K+1, 438): W.T + bias row, per-gate 146-col
    blocks (zero pad col). z_pad_bias=100 on the ih matrix makes the h'
    garbage slot compute to exactly 1.0."""
    K = W.shape[1]
    full = np.concatenate([W.T, b[None, :]], axis=0).astype(np.float32)
    out = np.zeros((K + 1, M3), np.float32)
    for g in range(3):
        out[:, g * GP: g * GP + G] = full[:, g * G: (g + 1) * G]
    out[K, GP + G] = z_pad_bias
    return out


def pack_inputs(inputs):
    f = lambda a: np.asarray(a, np.float32)
    wih0 = _gate_pack(f(inputs["Wih0"]), f(inputs["bih0"]), 100.0)  # (561, 438)
    ma = np.zeros((128, A_F), np.float32)
    for c in range(4):
        ma[:, c * M3:(c + 1) * M3] = wih0[c * 128:(c + 1) * 128, :]

    mb = np.zeros((C, B_F), np.float32)
    hn = f(inputs["hn"])
    for col, h in ((B_H0, hn[0]), (B_H1, hn[1])):
        v = np.append(h, np.float32(1.0)).reshape(2, C).T  # [73,2]
        mb[:, col:col + 4] = v[:, [0, 0, 1, 1]]            # paired
    mb[0:49, B_WIH0C4:B_WIH0C4 + M3] = wih0[512:561, :]
    for col, W, b, zb in (
        (B_WHH0, inputs["Whh0"], inputs["bhh0"], 0.0),
        (B_WIH1, inputs["Wih1"], inputs["bih1"], 100.0),
        (B_WHH1, inputs["Whh1"], inputs["bhh1"], 0.0),
    ):
        wt = _gate_pack(f(W), f(b), zb)                    # (146, 438)
        mb[:, col:col + M3] = wt[0:C, :]
        mb[:, col + M3:col + 2 * M3] = wt[C:2 * C, :]
    w2a = np.zeros((2 * C, H2 + 1), np.float32)
    w2a[0:G + 1, 0:H2] = np.concatenate(
        [f(inputs["W2a"]).T, f(inputs["b2a"])[None, :]], axis=0)
    w2a[G, H2] = 1.0                 # unit col -> l2h slot computes to 1.0
    mb[:, B_W2A:B_W2A + H2 + 1] = w2a[0:C, :]
    mb[:, B_W2A + H2 + 1:B_W2A + 2 * (H2 + 1)] = w2a[C:2 * C, :]
    w2b = np.concatenate([f(inputs["W2b"]).T, f(inputs["b2b"])[None, :]], axis=0)
    mb[0:H2 + 1, B_W2B:B_W2B + NO] = w2b

    mc = np.zeros((15, C_F), np.float32)
    mc[:, 0:H1] = np.concatenate(
        [f(inputs["W1"]).T, f(inputs["b1"])[None, :]], axis=0)
    mc[14, H1] = 1.0                 # unit col -> l1 slot computes to 1.0
    x_ext = np.concatenate([
        f(inputs["state_inno"]), f(inputs["obs_inno"]),
        f(inputs["diff_state"]), f(inputs["diff_obs"]), [np.float32(1.0)],
    ])
    mc[:, H1 + 1] = x_ext
    mc[:, H1 + 2] = x_ext
    return {"mega_a": ma, "mega_b1": np.ascontiguousarray(mb[:, 0:B_WIH1]),
            "mega_b2": np.ascontiguousarray(mb[:, B_WIH1:B_F]), "mega_c": mc}


def kernel(**inputs):
    nc = _get_nc()
    in_map = pack_inputs(inputs)
    res = bass_utils.run_bass_kernel_spmd(nc, [in_map], core_ids=[0], trace=TRACE)
    kernel.last_result = res
    return np.asarray(res.results[0]["out"], np.float32).reshape(X_DIM, Y_DIM)


# revision 20
# speedup vs baseline: 1.7703x; 1.4955x over previous
"""KalmanNet SLAM DNN forward pass on a single Trainium2 NeuronCore.

Network: x(14) -> Linear(560)+ReLU -> GRUCell(145) -> GRUCell(145)
         -> Linear(40)+ReLU -> Linear(10) -> reshape (5,2)

~1.8MB of fp32 weights, single sample => memory-bound; replicate on one
core (per sharding hint).

Matvecs run weights-stationary on the TensorEngine in fp32r (single-pass
fp32; ~1e-4-class relative error, far inside the scale-relative gate).
fp32r requires an even moving free dim, so every activation vector is
kept in duplicated column pairs ([K,2] rhs -> [M,2] psum) end to end.

Host-side numpy packs everything into three partition-major DRAM images
(per-partition contiguous runs of 2-14KB => near-line-rate SWDGE
descriptors, ~5% padding total), weights pre-transposed to [K, M]
layout, biases folded as an extra weight row against a constant-1.0
input element, GRU gates padded 145->146 so output chunks are uniform
73 partitions, and the z-gate pad-column bias set to 100 so the h'
garbage slot computes to exactly the 1.0 the next bias row needs.

Pointwise GRU math on VectorE, Sigmoid/Tanh on ScalarE (table-set load
pulled to t=0 by a dummy op), plus a dummy-matmul burst to warm the PE
clock during the DMA window.
"""

import numpy as np

import concourse.bacc as bacc
import concourse.mybir as mybir
import concourse.tile as tile
from concourse import bass_utils

F32 = mybir.dt.float32
F32R = mybir.dt.float32r
AF = mybir.ActivationFunctionType

X_DIM, Y_DIM = 5, 2
H1, H2 = 560, 40
G = 145          # GRU hidden size
C = 73           # partition chunk for the GRU state (2*73 = 146 = G+1)
GP = 2 * C       # per-gate padded column block
M3 = 3 * GP      # 438 padded gate columns
NO = X_DIM * Y_DIM

# megaB (73-partition image) column map
B_H0, B_H1 = 0, 4                      # h pairs [73,4]: (c0,c0,c1,c1)
B_WHH0 = 8                             # 2 x 438
B_WIH0C4 = B_WHH0 + 2 * M3             # [49,438]
B_WIH1 = B_WIH0C4 + M3                 # 2 x 438
B_WHH1 = B_WIH1 + 2 * M3               # 2 x 438
B_W2A = B_WHH1 + 2 * M3                # 2 x 41 (41st col makes the 1.0)
B_W2B = B_W2A + 2 * (H2 + 1)           # [41,10]
B_F = B_W2B + NO                       # 3164

A_F = 4 * M3                           # mega128: wih0 chunks c0..c3
C_F = H1 + 3                           # megaC: W1T(561, unit col) + x pair

TRACE = False
_BUILT = None


def _emit_gru(nc, pp, ab, name, wih_chunks, whh_chunks, h_sb,
              ptag_rz, ptag_ni, ptag_nh):
    """One GRU cell, everything in duplicated column pairs.
    *_chunks: (lhsT[K, 438], rhs[K, 2]); h_sb: [73,4] prev hidden pairs
    with slots (72, 2:4) = 1.0. Returns h' [73,4] F32R pairs."""
    ps_rz = pp.tile([C, 8], F32, tag=ptag_rz)   # (r_c0, r_c1, z_c0, z_c1) pairs
    ps_ni = pp.tile([C, 4], F32, tag=ptag_ni)
    ps_nh = pp.tile([C, 4], F32, tag=ptag_nh)

    nwi, nwh = len(wih_chunks), len(whh_chunks)
    rz_n = 4 * (nwh + nwi)
    rz_i = ni_i = nh_i = 0
    # gi first (its weights arrive first), gh accumulates on top
    for kc, (lhsT, rhs) in enumerate(wih_chunks):
        for g in (0, 1):
            for c in (0, 1):
                j = 2 * g + c
                nc.tensor.matmul(
                    ps_rz[:, 2 * j: 2 * j + 2],
                    lhsT[:, g * GP + C * c: g * GP + C * (c + 1)],
                    rhs, start=(rz_i == 0), stop=(rz_i == rz_n - 1))
                rz_i += 1
        for c in (0, 1):
            nc.tensor.matmul(
                ps_ni[:, 2 * c: 2 * c + 2],
                lhsT[:, 2 * GP + C * c: 2 * GP + C * (c + 1)],
                rhs, start=(ni_i == 0), stop=(ni_i == 2 * nwi - 1))
            ni_i += 1
    for kc, (lhsT, rhs) in enumerate(whh_chunks):
        for g in (0, 1):
            for c in (0, 1):
                j = 2 * g + c
                nc.tensor.matmul(
                    ps_rz[:, 2 * j: 2 * j + 2],
                    lhsT[:, g * GP + C * c: g * GP + C * (c + 1)],
                    rhs, start=(rz_i == 0), stop=(rz_i == rz_n - 1))
                rz_i += 1
        for c in (0, 1):
            nc.tensor.matmul(
                ps_nh[:, 2 * c: 2 * c + 2],
                lhsT[:, 2 * GP + C * c: 2 * GP + C * (c + 1)],
                rhs, start=(nh_i == 0), stop=(nh_i == 2 * nwh - 1))
            nh_i += 1

    # r,z = sigmoid(rz sums); n = tanh(i_n + r*h_n); h' = n + z*(h-n)
    rz = ab.tile([C, 8], F32, tag=f"{name}_rz")
    nc.scalar.activation(rz, ps_rz, AF.Sigmoid)
    t1 = ab.tile([C, 4], F32, tag=f"{name}_t1")
    nc.vector.tensor_mul(t1, rz[:, 0:4], ps_nh)
    nc.vector.tensor_add(t1, t1, ps_ni)
    n_sb = ab.tile([C, 4], F32, tag=f"{name}_n")
    nc.scalar.activation(n_sb, t1, AF.Tanh)
    d = ab.tile([C, 4], F32, tag=f"{name}_d")
    nc.vector.tensor_sub(d, h_sb.bitcast(F32), n_sb)
    nc.vector.tensor_mul(d, d, rz[:, 4:8])
    hp = ab.tile([C, 4], F32R, tag=f"{name}_hp")
    nc.vector.tensor_add(hp, n_sb, d)
    return hp


def _build():
    nc = bacc.Bacc("TRN2", num_devices=1, num_swdge_queues=4)

    d_a = nc.dram_tensor("mega_a", [128, A_F], F32R, kind="ExternalInput").ap()
    d_b1 = nc.dram_tensor("mega_b1", [128, B_WIH1], F32R, kind="ExternalInput").ap()
    d_b2 = nc.dram_tensor("mega_b2", [128, B_WHH1 - B_WIH1], F32R, kind="ExternalInput").ap()
    d_b3 = nc.dram_tensor("mega_b3", [128, B_F - B_WHH1], F32R, kind="ExternalInput").ap()
    d_c = nc.dram_tensor("mega_c", [15, C_F], F32R, kind="ExternalInput").ap()
    d_out = nc.dram_tensor("out", [1, NO], F32, kind="ExternalOutput").ap()

    with tile.TileContext(nc) as tc:
        with (
            tc.tile_pool(name="wp", bufs=1) as wp,
            tc.tile_pool(name="ab", bufs=1) as ab,
            tc.tile_pool(name="pp", bufs=1, space="PSUM") as pp,
        ):
            # ACT table warmup
            warm = ab.tile([1, 1], F32, tag="warm")
            nc.vector.memset(warm, 0.0)
            warm2 = ab.tile([1, 1], F32, tag="warm2")
            nc.scalar.activation(warm2, warm, AF.Sigmoid)
            nc.scalar.activation(warm2, warm2, AF.Tanh)

            # --- DMAs (SWDGE), priority order ---
            mc = wp.tile([15, C_F], F32R, tag="mc")
            nc.sync.dma_start(mc, d_c)        # tiny; separate HWDGE ring
            ma = wp.tile([128, A_F], F32R, tag="ma")
            nc.gpsimd.dma_start(ma, d_a)
            mb = wp.tile([128, B_F], F32R, tag="mb")
            nc.gpsimd.dma_start(mb[:, 0:B_WIH1], d_b1)
            nc.gpsimd.dma_start(mb[:, B_WIH1:B_WHH1], d_b2)
            nc.gpsimd.dma_start(mb[:, B_WHH1:B_F], d_b3)

            # --- PE warmup: dummy fp32 matmuls (~3.5us of HAM activity) ---
            wz = ab.tile([128, 128], F32, tag="wz")
            nc.vector.memset(wz, 0.0)
            ps_w = pp.tile([2, 128], F32, tag="pw")
            for i in range(8):
                nc.tensor.matmul(ps_w, wz[:, 0:2], wz,
                                 start=(i == 0), stop=(i == 7))

            # --- layer 1: l1 = relu(W1 @ x + b1), [128,10] paired cols ---
            x2 = mc[:, H1 + 1:H1 + 3]
            ps_l1 = pp.tile([128, 8], F32, tag="p0")
            for c in range(4):
                nc.tensor.matmul(ps_l1[:, 2 * c:2 * c + 2],
                                 mc[:, c * 128:(c + 1) * 128], x2,
                                 start=(c == 0), stop=(c == 3))
            ps_l1b = pp.tile([49, 2], F32, tag="p5")
            nc.tensor.matmul(ps_l1b, mc[:, 512:561], x2, start=True, stop=True)
            l1_sb = ab.tile([128, 10], F32R, tag="l1")
            nc.vector.tensor_scalar_max(l1_sb[:, 0:8], ps_l1, 0.0)
            nc.vector.tensor_scalar_max(l1_sb[0:49, 8:10], ps_l1b, 0.0)

            h0_sb = mb[0:C, B_H0:B_H0 + 4]
            h1_sb = mb[0:C, B_H1:B_H1 + 4]

            # --- GRU 0 ---
            wih0_chunks = [
                (ma[:, c * M3:(c + 1) * M3], l1_sb[:, 2 * c:2 * c + 2])
                for c in range(4)
            ] + [
                (mb[0:49, B_WIH0C4:B_WIH0C4 + M3], l1_sb[0:49, 8:10])
            ]
            whh0_chunks = [
                (mb[0:C, B_WHH0 + c * M3: B_WHH0 + (c + 1) * M3],
                 h0_sb[:, 2 * c:2 * c + 2])
                for c in range(2)
            ]
            hp0 = _emit_gru(nc, pp, ab, "g0", wih0_chunks, whh0_chunks, h0_sb,
                            "p1", "p2", "p3")

            # --- GRU 1 ---
            wih1_chunks = [
                (mb[0:C, B_WIH1 + c * M3: B_WIH1 + (c + 1) * M3],
                 hp0[:, 2 * c:2 * c + 2])
                for c in range(2)
            ]
            whh1_chunks = [
                (mb[0:C, B_WHH1 + c * M3: B_WHH1 + (c + 1) * M3],
                 h1_sb[:, 2 * c:2 * c + 2])
                for c in range(2)
            ]
            hp1 = _emit_gru(nc, pp, ab, "g1", wih1_chunks, whh1_chunks, h1_sb,
                            "p0", "p1", "p2")

            # --- l2 ---
            ps_a = pp.tile([H2 + 1, 2], F32, tag="p3")
            for c in range(2):
                nc.tensor.matmul(
                    ps_a, mb[0:C, B_W2A + c * (H2 + 1): B_W2A + (c + 1) * (H2 + 1)],
                    hp1[:, 2 * c:2 * c + 2], start=(c == 0), stop=(c == 1))
            l2h = ab.tile([H2 + 1, 2], F32R, tag="l2h")
            nc.vector.tensor_scalar_max(l2h, ps_a, 0.0)
            ps_o = pp.tile([1, NO], F32, tag="p4")
            nc.tensor.matmul(ps_o, l2h[:, 0:1],
                             mb[0:H2 + 1, B_W2B:B_W2B + NO],
                             start=True, stop=True)
            out_sb = ab.tile([1, NO], F32, tag="out_sb")
            nc.vector.tensor_copy(out_sb, ps_o)
            nc.sync.dma_start(d_out, out_sb)

    nc.compile()
    return nc


def _get_nc():
    global _BUILT
    if _BUILT is None:
        _BUILT = _build()
    return _BUILT


def _gate_pack(W, b, z_pad_bias=0.0):
    """W:(435,K), b:(435,) -> (K+1, 438): W.T + bias row, per-gate 146-col
    blocks (zero pad col). z_pad_bias=100 on the ih matrix makes the h'
    garbage slot compute to exactly 1.0."""
    K = W.shape[1]
    full = np.concatenate([W.T, b[None, :]], axis=0).astype(np.float32)
    out = np.zeros((K + 1, M3), np.float32)
    for g in range(3):
        out[:, g * GP: g * GP + G] = full[:, g * G: (g + 1) * G]
    out[K, GP + G] = z_pad_bias
    return out


def pack_inputs(inputs):
    f = lambda a: np.asarray(a, np.float32)
    wih0 = _gate_pack(f(inputs["Wih0"]), f(inputs["bih0"]), 100.0)  # (561, 438)
    ma = np.zeros((128, A_F), np.float32)
    for c in range(4):
        ma[:, c * M3:(c + 1) * M3] = wih0[c * 128:(c + 1) * 128, :]

    mb = np.zeros((128, B_F), np.float32)
    hn = f(inputs["hn"])
    for col, h in ((B_H0, hn[0]), (B_H1, hn[1])):
        v = np.append(h, np.float32(1.0)).reshape(2, C).T  # [73,2]
        mb[0:C, col:col + 4] = v[:, [0, 0, 1, 1]]          # paired
    mb[0:49, B_WIH0C4:B_WIH0C4 + M3] = wih0[512:561, :]
    for col, W, b, zb in (
        (B_WHH0, inputs["Whh0"], inputs["bhh0"], 0.0),
        (B_WIH1, inputs["Wih1"], inputs["bih1"], 100.0),
        (B_WHH1, inputs["Whh1"], inputs["bhh1"], 0.0),
    ):
        wt = _gate_pack(f(W), f(b), zb)                    # (146, 438)
        mb[0:C, col:col + M3] = wt[0:C, :]
        mb[0:C, col + M3:col + 2 * M3] = wt[C:2 * C, :]
    w2a = np.zeros((2 * C, H2 + 1), np.float32)
    w2a[0:G + 1, 0:H2] = np.concatenate(
        [f(inputs["W2a"]).T, f(inputs["b2a"])[None, :]], axis=0)
    w2a[G, H2] = 1.0                 # unit col -> l2h slot computes to 1.0
    mb[0:C, B_W2A:B_W2A + H2 + 1] = w2a[0:C, :]
    mb[0:C, B_W2A + H2 + 1:B_W2A + 2 * (H2 + 1)] = w2a[C:2 * C, :]
    w2b = np.concatenate([f(inputs["W2b"]).T, f(inputs["b2b"])[None, :]], axis=0)
    mb[0:H2 + 1, B_W2B:B_W2B + NO] = w2b

    mc = np.zeros((15, C_F), np.float32)
    mc[:, 0:H1] = np.concatenate(
        [f(inputs["W1"]).T, f(inputs["b1"])[None, :]], axis=0)
    mc[14, H1] = 1.0                 # unit col -> l1 slot computes to 1.0
    x_ext = np.concatenate([
        f(inputs["state_inno"]), f(inputs["obs_inno"]),
        f(inputs["diff_state"]), f(inputs["diff_obs"]), [np.float32(1.0)],
    ])
    mc[:, H1 + 1] = x_ext
    mc[:, H1 + 2] = x_ext
    return {"mega_a": ma,
            "mega_b1": np.ascontiguousarray(mb[:, 0:B_WIH1]),
            "mega_b2": np.ascontiguousarray(mb[:, B_WIH1:B_WHH1]),
            "mega_b3": np.ascontiguousarray(mb[:, B_WHH1:B_F]),
            "mega_c": mc}


def kernel(**inputs):
    nc = _get_nc()
    in_map = pack_inputs(inputs)
    res = bass_utils.run_bass_kernel_spmd(nc, [in_map], core_ids=[0], trace=TRACE)
    kernel.last_result = res
    return np.asarray(res.results[0]["out"], np.float32).reshape(X_DIM, Y_DIM)


# revision 21
# speedup vs baseline: 1.7770x; 1.0038x over previous
"""KalmanNet SLAM DNN forward pass on a single Trainium2 NeuronCore.

Network: x(14) -> Linear(560)+ReLU -> GRUCell(145) -> GRUCell(145)
         -> Linear(40)+ReLU -> Linear(10) -> reshape (5,2)

~1.8MB of fp32 weights, single sample => memory-bound; replicate on one
core (per sharding hint).

Matvecs run weights-stationary on the TensorEngine in fp32r (single-pass
fp32; ~1e-4-class relative error, far inside the scale-relative gate).
fp32r requires an even moving free dim, so every activation vector is
kept in duplicated column pairs ([K,2] rhs -> [M,2] psum) end to end.

Host-side numpy packs everything into three partition-major DRAM images
(per-partition contiguous runs of 2-14KB => near-line-rate SWDGE
descriptors, ~5% padding total), weights pre-transposed to [K, M]
layout, biases folded as an extra weight row against a constant-1.0
input element, GRU gates padded 145->146 so output chunks are uniform
73 partitions, and the z-gate pad-column bias set to 100 so the h'
garbage slot computes to exactly the 1.0 the next bias row needs.

Pointwise GRU math on VectorE, Sigmoid/Tanh on ScalarE (table-set load
pulled to t=0 by a dummy op), plus a dummy-matmul burst to warm the PE
clock during the DMA window.
"""

import numpy as np

import concourse.bacc as bacc
import concourse.mybir as mybir
import concourse.tile as tile
from concourse import bass_utils

F32 = mybir.dt.float32
F32R = mybir.dt.float32r
AF = mybir.ActivationFunctionType

X_DIM, Y_DIM = 5, 2
H1, H2 = 560, 40
G = 145          # GRU hidden size
C = 73           # partition chunk for the GRU state (2*73 = 146 = G+1)
GP = 2 * C       # per-gate padded column block
M3 = 3 * GP      # 438 padded gate columns
NO = X_DIM * Y_DIM

# megaB (73-partition image) column map
B_H0, B_H1 = 0, 4                      # h pairs [73,4]: (c0,c0,c1,c1)
B_WHH0 = 8                             # 2 x 438
B_WIH0C4 = B_WHH0 + 2 * M3             # [49,438]
B_WIH1 = B_WIH0C4 + M3                 # 2 x 438
B_WHH1 = B_WIH1 + 2 * M3               # 2 x 438
B_W2A = B_WHH1 + 2 * M3                # 2 x 41 (41st col makes the 1.0)
B_W2B = B_W2A + 2 * (H2 + 1)           # [41,10]
B_F = B_W2B + NO                       # 3164

A_F = 4 * M3                           # mega128: wih0 chunks c0..c3
C_F = H1 + 3                           # megaC: W1T(561, unit col) + x pair

TRACE = False
_BUILT = None


def _emit_gru(nc, pp, ab, name, wih_chunks, whh_chunks, h_sb,
              ptag_rz, ptag_ni, ptag_nh):
    """One GRU cell, everything in duplicated column pairs.
    *_chunks: (lhsT[K, 438], rhs[K, 2]); h_sb: [73,4] prev hidden pairs
    with slots (72, 2:4) = 1.0. Returns h' [73,4] F32R pairs."""
    ps_rz = pp.tile([C, 8], F32, tag=ptag_rz)   # (r_c0, r_c1, z_c0, z_c1) pairs
    ps_ni = pp.tile([C, 4], F32, tag=ptag_ni)
    ps_nh = pp.tile([C, 4], F32, tag=ptag_nh)

    # One psum tile at a time (single accumulation bank phase each), r/z
    # first so the sigmoid overlaps the n-gate matmuls.
    rz_n = 4 * (len(wih_chunks) + len(whh_chunks))
    rz_i = 0
    for lhsT, rhs in wih_chunks + whh_chunks:
        for g in (0, 1):
            for c in (0, 1):
                j = 2 * g + c
                nc.tensor.matmul(
                    ps_rz[:, 2 * j: 2 * j + 2],
                    lhsT[:, g * GP + C * c: g * GP + C * (c + 1)],
                    rhs, start=(rz_i == 0), stop=(rz_i == rz_n - 1))
                rz_i += 1
    for chunks, ps in ((whh_chunks, ps_nh), (wih_chunks, ps_ni)):
        n_i = 0
        for lhsT, rhs in chunks:
            for c in (0, 1):
                nc.tensor.matmul(
                    ps[:, 2 * c: 2 * c + 2],
                    lhsT[:, 2 * GP + C * c: 2 * GP + C * (c + 1)],
                    rhs, start=(n_i == 0), stop=(n_i == 2 * len(chunks) - 1))
                n_i += 1

    # r,z = sigmoid(rz sums); n = tanh(i_n + r*h_n); h' = n + z*(h-n)
    rz = ab.tile([C, 8], F32, tag=f"{name}_rz")
    nc.scalar.activation(rz, ps_rz, AF.Sigmoid)
    t1 = ab.tile([C, 4], F32, tag=f"{name}_t1")
    nc.vector.tensor_mul(t1, rz[:, 0:4], ps_nh)
    nc.vector.tensor_add(t1, t1, ps_ni)
    n_sb = ab.tile([C, 4], F32, tag=f"{name}_n")
    nc.scalar.activation(n_sb, t1, AF.Tanh)
    d = ab.tile([C, 4], F32, tag=f"{name}_d")
    nc.vector.tensor_sub(d, h_sb.bitcast(F32), n_sb)
    nc.vector.tensor_mul(d, d, rz[:, 4:8])
    hp = ab.tile([C, 4], F32R, tag=f"{name}_hp")
    nc.vector.tensor_add(hp, n_sb, d)
    return hp


def _build():
    nc = bacc.Bacc("TRN2", num_devices=1, num_swdge_queues=4)

    d_a = nc.dram_tensor("mega_a", [128, A_F], F32R, kind="ExternalInput").ap()
    d_b1 = nc.dram_tensor("mega_b1", [128, B_WIH1], F32R, kind="ExternalInput").ap()
    d_b2 = nc.dram_tensor("mega_b2", [128, B_WHH1 - B_WIH1], F32R, kind="ExternalInput").ap()
    d_b3 = nc.dram_tensor("mega_b3", [128, B_F - B_WHH1], F32R, kind="ExternalInput").ap()
    d_c = nc.dram_tensor("mega_c", [15, C_F], F32R, kind="ExternalInput").ap()
    d_out = nc.dram_tensor("out", [1, NO], F32, kind="ExternalOutput").ap()

    with tile.TileContext(nc) as tc:
        with (
            tc.tile_pool(name="wp", bufs=1) as wp,
            tc.tile_pool(name="ab", bufs=1) as ab,
            tc.tile_pool(name="pp", bufs=1, space="PSUM") as pp,
        ):
            # ACT table warmup
            warm = ab.tile([1, 1], F32, tag="warm")
            nc.vector.memset(warm, 0.0)
            warm2 = ab.tile([1, 1], F32, tag="warm2")
            nc.scalar.activation(warm2, warm, AF.Sigmoid)
            nc.scalar.activation(warm2, warm2, AF.Tanh)

            # --- DMAs (SWDGE), priority order ---
            mc = wp.tile([15, C_F], F32R, tag="mc")
            nc.sync.dma_start(mc, d_c)        # tiny; separate HWDGE ring
            ma = wp.tile([128, A_F], F32R, tag="ma")
            nc.gpsimd.dma_start(ma, d_a)
            mb = wp.tile([128, B_F], F32R, tag="mb")
            nc.gpsimd.dma_start(mb[:, 0:B_WIH1], d_b1)
            nc.gpsimd.dma_start(mb[:, B_WIH1:B_WHH1], d_b2)
            nc.gpsimd.dma_start(mb[:, B_WHH1:B_F], d_b3)

            # --- PE warmup: dummy fp32 matmuls (~3.5us of HAM activity) ---
            wz = ab.tile([128, 128], F32, tag="wz")
            nc.vector.memset(wz, 0.0)
            ps_w = pp.tile([2, 128], F32, tag="pw")
            for i in range(8):
                nc.tensor.matmul(ps_w, wz[:, 0:2], wz,
                                 start=(i == 0), stop=(i == 7))

            # --- layer 1: l1 = relu(W1 @ x + b1), [128,10] paired cols ---
            x2 = mc[:, H1 + 1:H1 + 3]
            ps_l1 = pp.tile([128, 8], F32, tag="p0")
            for c in range(4):
                nc.tensor.matmul(ps_l1[:, 2 * c:2 * c + 2],
                                 mc[:, c * 128:(c + 1) * 128], x2,
                                 start=(c == 0), stop=(c == 3))
            ps_l1b = pp.tile([49, 2], F32, tag="p5")
            nc.tensor.matmul(ps_l1b, mc[:, 512:561], x2, start=True, stop=True)
            l1_sb = ab.tile([128, 10], F32R, tag="l1")
            nc.vector.tensor_scalar_max(l1_sb[:, 0:8], ps_l1, 0.0)
            nc.vector.tensor_scalar_max(l1_sb[0:49, 8:10], ps_l1b, 0.0)

            h0_sb = mb[0:C, B_H0:B_H0 + 4]
            h1_sb = mb[0:C, B_H1:B_H1 + 4]

            # --- GRU 0 ---
            wih0_chunks = [
                (ma[:, c * M3:(c + 1) * M3], l1_sb[:, 2 * c:2 * c + 2])
                for c in range(4)
            ] + [
                (mb[0:49, B_WIH0C4:B_WIH0C4 + M3], l1_sb[0:49, 8:10])
            ]
            whh0_chunks = [
                (mb[0:C, B_WHH0 + c * M3: B_WHH0 + (c + 1) * M3],
                 h0_sb[:, 2 * c:2 * c + 2])
                for c in range(2)
            ]
            hp0 = _emit_gru(nc, pp, ab, "g0", wih0_chunks, whh0_chunks, h0_sb,
                            "p1", "p2", "p3")

            # --- GRU 1 ---
            wih1_chunks = [
                (mb[0:C, B_WIH1 + c * M3: B_WIH1 + (c + 1) * M3],
                 hp0[:, 2 * c:2 * c + 2])
                for c in range(2)
            ]
            whh1_chunks = [
                (mb[0:C, B_WHH1 + c * M3: B_WHH1 + (c + 1) * M3],
                 h1_sb[:, 2 * c:2 * c + 2])
                for c in range(2)
            ]
            hp1 = _emit_gru(nc, pp, ab, "g1", wih1_chunks, whh1_chunks, h1_sb,
                            "p0", "p1", "p2")

            # --- l2 ---
            ps_a = pp.tile([H2 + 1, 2], F32, tag="p3")
            for c in range(2):
                nc.tensor.matmul(
                    ps_a, mb[0:C, B_W2A + c * (H2 + 1): B_W2A + (c + 1) * (H2 + 1)],
                    hp1[:, 2 * c:2 * c + 2], start=(c == 0), stop=(c == 1))
            l2h = ab.tile([H2 + 1, 2], F32R, tag="l2h")
            nc.vector.tensor_scalar_max(l2h, ps_a, 0.0)
            ps_o = pp.tile([1, NO], F32, tag="p4")
            nc.tensor.matmul(ps_o, l2h[:, 0:1],
                             mb[0:H2 + 1, B_W2B:B_W2B + NO],
                             start=True, stop=True)
            out_sb = ab.tile([1, NO], F32, tag="out_sb")
            nc.vector.tensor_copy(out_sb, ps_o)
            nc.sync.dma_start(d_out, out_sb)

    nc.compile()
    return nc


def _get_nc():
    global _BUILT
    if _BUILT is None:
        _BUILT = _build()
    return _BUILT


def _gate_pack(W, b, z_pad_bias=0.0):
    """W:(435,K), b:(435,) -> (K+1, 438): W.T + bias row, per-gate 146-col
    blocks (zero pad col). z_pad_bias=100 on the ih matrix makes the h'
    garbage slot compute to exactly 1.0."""
    K = W.shape[1]
    full = np.concatenate([W.T, b[None, :]], axis=0).astype(np.float32)
    out = np.zeros((K + 1, M3), np.float32)
    for g in range(3):
        out[:, g * GP: g * GP + G] = full[:, g * G: (g + 1) * G]
    out[K, GP + G] = z_pad_bias
    return out


def pack_inputs(inputs):
    f = lambda a: np.asarray(a, np.float32)
    wih0 = _gate_pack(f(inputs["Wih0"]), f(inputs["bih0"]), 100.0)  # (561, 438)
    ma = np.zeros((128, A_F), np.float32)
    for c in range(4):
        ma[:, c * M3:(c + 1) * M3] = wih0[c * 128:(c + 1) * 128, :]

    mb = np.zeros((128, B_F), np.float32)
    hn = f(inputs["hn"])
    for col, h in ((B_H0, hn[0]), (B_H1, hn[1])):
        v = np.append(h, np.float32(1.0)).reshape(2, C).T  # [73,2]
        mb[0:C, col:col + 4] = v[:, [0, 0, 1, 1]]          # paired
    mb[0:49, B_WIH0C4:B_WIH0C4 + M3] = wih0[512:561, :]
    for col, W, b, zb in (
        (B_WHH0, inputs["Whh0"], inputs["bhh0"], 0.0),
        (B_WIH1, inputs["Wih1"], inputs["bih1"], 100.0),
        (B_WHH1, inputs["Whh1"], inputs["bhh1"], 0.0),
    ):
        wt = _gate_pack(f(W), f(b), zb)                    # (146, 438)
        mb[0:C, col:col + M3] = wt[0:C, :]
        mb[0:C, col + M3:col + 2 * M3] = wt[C:2 * C, :]
    w2a = np.zeros((2 * C, H2 + 1), np.float32)
    w2a[0:G + 1, 0:H2] = np.concatenate(
        [f(inputs["W2a"]).T, f(inputs["b2a"])[None, :]], axis=0)
    w2a[G, H2] = 1.0                 # unit col -> l2h slot computes to 1.0
    mb[0:C, B_W2A:B_W2A + H2 + 1] = w2a[0:C, :]
    mb[0:C, B_W2A + H2 + 1:B_W2A + 2 * (H2 + 1)] = w2a[C:2 * C, :]
    w2b = np.concatenate([f(inputs["W2b"]).T, f(inputs["b2b"])[None, :]], axis=0)
    mb[0:H2 + 1, B_W2B:B_W2B + NO] = w2b

    mc = np.zeros((15, C_F), np.float32)
    mc[:, 0:H1] = np.concatenate(
        [f(inputs["W1"]).T, f(inputs["b1"])[None, :]], axis=0)
    mc[14, H1] = 1.0                 # unit col -> l1 slot computes to 1.0
    x_ext = np.concatenate([
        f(inputs["state_inno"]), f(inputs["obs_inno"]),
        f(inputs["diff_state"]), f(inputs["diff_obs"]), [np.float32(1.0)],
    ])
    mc[:, H1 + 1] = x_ext
    mc[:, H1 + 2] = x_ext
    return {"mega_a": ma,
            "mega_b1": np.ascontiguousarray(mb[:, 0:B_WIH1]),
            "mega_b2": np.ascontiguousarray(mb[:, B_WIH1:B_WHH1]),
            "mega_b3": np.ascontiguousarray(mb[:, B_WHH1:B_F]),
            "mega_c": mc}


def kernel(**inputs):
    nc = _get_nc()
    in_map = pack_inputs(inputs)
    res = bass_utils.run_bass_kernel_spmd(nc, [in_map], core_ids=[0], trace=TRACE)
    kernel.last_result = res
    return np.asarray(res.results[0]["out"], np.float32).reshape(X_DIM, Y_DIM)


# revision 22
# speedup vs baseline: 1.8522x; 1.0423x over previous
"""KalmanNet SLAM DNN forward pass on a single Trainium2 NeuronCore.

Network: x(14) -> Linear(560)+ReLU -> GRUCell(145) -> GRUCell(145)
         -> Linear(40)+ReLU -> Linear(10) -> reshape (5,2)

~1.8MB of fp32 weights, single sample => memory-bound; replicate on one
core (per sharding hint).

Matvecs run weights-stationary on the TensorEngine in fp32r (single-pass
fp32; ~1e-4-class relative error, far inside the scale-relative gate).
fp32r requires an even moving free dim, so every activation vector is
kept in duplicated column pairs ([K,2] rhs -> [M,2] psum) end to end.

Host-side numpy packs everything into three partition-major DRAM images
(per-partition contiguous runs of 2-14KB => near-line-rate SWDGE
descriptors, ~5% padding total), weights pre-transposed to [K, M]
layout, biases folded as an extra weight row against a constant-1.0
input element, GRU gates padded 145->146 so output chunks are uniform
73 partitions, and the z-gate pad-column bias set to 100 so the h'
garbage slot computes to exactly the 1.0 the next bias row needs.

Pointwise GRU math on VectorE, Sigmoid/Tanh on ScalarE (table-set load
pulled to t=0 by a dummy op), plus a dummy-matmul burst to warm the PE
clock during the DMA window.
"""

import numpy as np

import concourse.bacc as bacc
import concourse.mybir as mybir
import concourse.tile as tile
from concourse import bass_utils

F32 = mybir.dt.float32
F32R = mybir.dt.float32r
AF = mybir.ActivationFunctionType

X_DIM, Y_DIM = 5, 2
H1, H2 = 560, 40
G = 145          # GRU hidden size
C = 73           # partition chunk for the GRU state (2*73 = 146 = G+1)
GP = 2 * C       # per-gate padded column block
M3 = 3 * GP      # 438 padded gate columns
NO = X_DIM * Y_DIM

# megaB (73-partition image) column map
B_H0, B_H1 = 0, 4                      # h pairs [73,4]: (c0,c0,c1,c1)
B_WHH0 = 8                             # 2 x 438
B_WIH0C4 = B_WHH0 + 2 * M3             # [49,438]
B_WIH1 = B_WIH0C4 + M3                 # 2 x 438
B_WHH1 = B_WIH1 + 2 * M3               # 2 x 438
B_W2A = B_WHH1 + 2 * M3                # 2 x 41 (41st col makes the 1.0)
B_W2B = B_W2A + 2 * (H2 + 1)           # [41,10]
B_F = B_W2B + NO                       # 3164

A_F = 4 * M3                           # mega128: wih0 chunks c0..c3
C_F = H1 + 3                           # megaC: W1T(561, unit col) + x pair

TRACE = False
_BUILT = None


def _emit_gru(nc, pp, ab, name, wih_chunks, whh_chunks, h_sb,
              ptag_rz, ptag_ni, ptag_nh, hh_first=False):
    """One GRU cell, everything in duplicated column pairs.
    *_chunks: (lhsT[K, 438], rhs[K, 2]); h_sb: [73,4] prev hidden pairs
    with slots (72, 2:4) = 1.0. Returns h' [73,4] F32R pairs."""
    ps_rz = pp.tile([C, 8], F32, tag=ptag_rz)   # (r_c0, r_c1, z_c0, z_c1) pairs
    ps_ni = pp.tile([C, 4], F32, tag=ptag_ni)
    ps_nh = pp.tile([C, 4], F32, tag=ptag_nh)

    # One psum tile at a time (single accumulation bank phase each), r/z
    # first so the sigmoid overlaps the n-gate matmuls.
    rz_n = 4 * (len(wih_chunks) + len(whh_chunks))
    rz_i = 0
    rz_order = (whh_chunks + wih_chunks) if hh_first else (wih_chunks + whh_chunks)
    for lhsT, rhs in rz_order:
        for g in (0, 1):
            for c in (0, 1):
                j = 2 * g + c
                nc.tensor.matmul(
                    ps_rz[:, 2 * j: 2 * j + 2],
                    lhsT[:, g * GP + C * c: g * GP + C * (c + 1)],
                    rhs, start=(rz_i == 0), stop=(rz_i == rz_n - 1))
                rz_i += 1
    for chunks, ps in ((whh_chunks, ps_nh), (wih_chunks, ps_ni)):
        n_i = 0
        for lhsT, rhs in chunks:
            for c in (0, 1):
                nc.tensor.matmul(
                    ps[:, 2 * c: 2 * c + 2],
                    lhsT[:, 2 * GP + C * c: 2 * GP + C * (c + 1)],
                    rhs, start=(n_i == 0), stop=(n_i == 2 * len(chunks) - 1))
                n_i += 1

    # r,z = sigmoid(rz sums); n = tanh(i_n + r*h_n); h' = n + z*(h-n)
    rz = ab.tile([C, 8], F32, tag=f"{name}_rz")
    nc.scalar.activation(rz, ps_rz, AF.Sigmoid)
    t1 = ab.tile([C, 4], F32, tag=f"{name}_t1")
    nc.vector.tensor_mul(t1, rz[:, 0:4], ps_nh)
    nc.vector.tensor_add(t1, t1, ps_ni)
    n_sb = ab.tile([C, 4], F32, tag=f"{name}_n")
    nc.scalar.activation(n_sb, t1, AF.Tanh)
    d = ab.tile([C, 4], F32, tag=f"{name}_d")
    nc.vector.tensor_sub(d, h_sb.bitcast(F32), n_sb)
    nc.vector.tensor_mul(d, d, rz[:, 4:8])
    hp = ab.tile([C, 4], F32R, tag=f"{name}_hp")
    nc.vector.tensor_add(hp, n_sb, d)
    return hp


def _build():
    nc = bacc.Bacc("TRN2", num_devices=1, num_swdge_queues=4)

    d_a = nc.dram_tensor("mega_a", [128, A_F], F32R, kind="ExternalInput").ap()
    d_b1 = nc.dram_tensor("mega_b1", [128, B_WIH1], F32R, kind="ExternalInput").ap()
    d_b2 = nc.dram_tensor("mega_b2", [128, B_WHH1 - B_WIH1], F32R, kind="ExternalInput").ap()
    d_b3 = nc.dram_tensor("mega_b3", [128, B_F - B_WHH1], F32R, kind="ExternalInput").ap()
    d_c = nc.dram_tensor("mega_c", [15, C_F], F32R, kind="ExternalInput").ap()
    d_out = nc.dram_tensor("out", [1, NO], F32, kind="ExternalOutput").ap()

    with tile.TileContext(nc) as tc:
        with (
            tc.tile_pool(name="wp", bufs=1) as wp,
            tc.tile_pool(name="ab", bufs=1) as ab,
            tc.tile_pool(name="pp", bufs=1, space="PSUM") as pp,
        ):
            # ACT table warmup
            warm = ab.tile([1, 1], F32, tag="warm")
            nc.vector.memset(warm, 0.0)
            warm2 = ab.tile([1, 1], F32, tag="warm2")
            nc.scalar.activation(warm2, warm, AF.Sigmoid)
            nc.scalar.activation(warm2, warm2, AF.Tanh)

            # --- DMAs (SWDGE), priority order ---
            mc = wp.tile([15, C_F], F32R, tag="mc")
            nc.sync.dma_start(mc, d_c)        # tiny; separate HWDGE ring
            ma = wp.tile([128, A_F], F32R, tag="ma")
            nc.gpsimd.dma_start(ma, d_a)
            mb = wp.tile([128, B_F], F32R, tag="mb")
            nc.gpsimd.dma_start(mb[:, 0:B_WIH1], d_b1)
            nc.gpsimd.dma_start(mb[:, B_WIH1:B_WHH1], d_b2)
            nc.gpsimd.dma_start(mb[:, B_WHH1:B_F], d_b3)

            # --- PE warmup: dummy fp32 matmuls (~3.5us of HAM activity) ---
            wz = ab.tile([128, 128], F32, tag="wz")
            nc.vector.memset(wz, 0.0)
            ps_w = pp.tile([2, 128], F32, tag="pw")
            for i in range(8):
                nc.tensor.matmul(ps_w, wz[:, 0:2], wz,
                                 start=(i == 0), stop=(i == 7))

            # --- layer 1: l1 = relu(W1 @ x + b1), [128,10] paired cols ---
            x2 = mc[:, H1 + 1:H1 + 3]
            ps_l1 = pp.tile([128, 8], F32, tag="p0")
            for c in range(4):
                nc.tensor.matmul(ps_l1[:, 2 * c:2 * c + 2],
                                 mc[:, c * 128:(c + 1) * 128], x2,
                                 start=(c == 0), stop=(c == 3))
            ps_l1b = pp.tile([49, 2], F32, tag="p5")
            nc.tensor.matmul(ps_l1b, mc[:, 512:561], x2, start=True, stop=True)
            l1_sb = ab.tile([128, 10], F32R, tag="l1")
            nc.vector.tensor_scalar_max(l1_sb[:, 0:8], ps_l1, 0.0)
            nc.vector.tensor_scalar_max(l1_sb[0:49, 8:10], ps_l1b, 0.0)

            h0_sb = mb[0:C, B_H0:B_H0 + 4]
            h1_sb = mb[0:C, B_H1:B_H1 + 4]

            # --- GRU 0 ---
            wih0_chunks = [
                (ma[:, c * M3:(c + 1) * M3], l1_sb[:, 2 * c:2 * c + 2])
                for c in range(4)
            ] + [
                (mb[0:49, B_WIH0C4:B_WIH0C4 + M3], l1_sb[0:49, 8:10])
            ]
            whh0_chunks = [
                (mb[0:C, B_WHH0 + c * M3: B_WHH0 + (c + 1) * M3],
                 h0_sb[:, 2 * c:2 * c + 2])
                for c in range(2)
            ]
            hp0 = _emit_gru(nc, pp, ab, "g0", wih0_chunks, whh0_chunks, h0_sb,
                            "p1", "p2", "p3")

            # --- GRU 1 ---
            wih1_chunks = [
                (mb[0:C, B_WIH1 + c * M3: B_WIH1 + (c + 1) * M3],
                 hp0[:, 2 * c:2 * c + 2])
                for c in range(2)
            ]
            whh1_chunks = [
                (mb[0:C, B_WHH1 + c * M3: B_WHH1 + (c + 1) * M3],
                 h1_sb[:, 2 * c:2 * c + 2])
                for c in range(2)
            ]
            hp1 = _emit_gru(nc, pp, ab, "g1", wih1_chunks, whh1_chunks, h1_sb,
                            "p0", "p1", "p2", hh_first=True)

            # --- l2 ---
            ps_a = pp.tile([H2 + 1, 2], F32, tag="p3")
            for c in range(2):
                nc.tensor.matmul(
                    ps_a, mb[0:C, B_W2A + c * (H2 + 1): B_W2A + (c + 1) * (H2 + 1)],
                    hp1[:, 2 * c:2 * c + 2], start=(c == 0), stop=(c == 1))
            l2h = ab.tile([H2 + 1, 2], F32R, tag="l2h")
            nc.vector.tensor_scalar_max(l2h, ps_a, 0.0)
            ps_o = pp.tile([1, NO], F32, tag="p4")
            nc.tensor.matmul(ps_o, l2h[:, 0:1],
                             mb[0:H2 + 1, B_W2B:B_W2B + NO],
                             start=True, stop=True)
            out_sb = ab.tile([1, NO], F32, tag="out_sb")
            nc.vector.tensor_copy(out_sb, ps_o)
            nc.sync.dma_start(d_out, out_sb)

    nc.compile()
    return nc


def _get_nc():
    global _BUILT
    if _BUILT is None:
        _BUILT = _build()
    return _BUILT


def _gate_pack(W, b, z_pad_bias=0.0):
    """W:(435,K), b:(435,) -> (K+1, 438): W.T + bias row, per-gate 146-col
    blocks (zero pad col). z_pad_bias=100 on the ih matrix makes the h'
    garbage slot compute to exactly 1.0."""
    K = W.shape[1]
    full = np.concatenate([W.T, b[None, :]], axis=0).astype(np.float32)
    out = np.zeros((K + 1, M3), np.float32)
    for g in range(3):
        out[:, g * GP: g * GP + G] = full[:, g * G: (g + 1) * G]
    out[K, GP + G] = z_pad_bias
    return out


def pack_inputs(inputs):
    f = lambda a: np.asarray(a, np.float32)
    wih0 = _gate_pack(f(inputs["Wih0"]), f(inputs["bih0"]), 100.0)  # (561, 438)
    ma = np.zeros((128, A_F), np.float32)
    for c in range(4):
        ma[:, c * M3:(c + 1) * M3] = wih0[c * 128:(c + 1) * 128, :]

    mb = np.zeros((128, B_F), np.float32)
    hn = f(inputs["hn"])
    for col, h in ((B_H0, hn[0]), (B_H1, hn[1])):
        v = np.append(h, np.float32(1.0)).reshape(2, C).T  # [73,2]
        mb[0:C, col:col + 4] = v[:, [0, 0, 1, 1]]          # paired
    mb[0:49, B_WIH0C4:B_WIH0C4 + M3] = wih0[512:561, :]
    for col, W, b, zb in (
        (B_WHH0, inputs["Whh0"], inputs["bhh0"], 0.0),
        (B_WIH1, inputs["Wih1"], inputs["bih1"], 100.0),
        (B_WHH1, inputs["Whh1"], inputs["bhh1"], 0.0),
    ):
        wt = _gate_pack(f(W), f(b), zb)                    # (146, 438)
        mb[0:C, col:col + M3] = wt[0:C, :]
        mb[0:C, col + M3:col + 2 * M3] = wt[C:2 * C, :]
    w2a = np.zeros((2 * C, H2 + 1), np.float32)
    w2a[0:G + 1, 0:H2] = np.concatenate(
        [f(inputs["W2a"]).T, f(inputs["b2a"])[None, :]], axis=0)
    w2a[G, H2] = 1.0                 # unit col -> l2h slot computes to 1.0
    mb[0:C, B_W2A:B_W2A + H2 + 1] = w2a[0:C, :]
    mb[0:C, B_W2A + H2 + 1:B_W2A + 2 * (H2 + 1)] = w2a[C:2 * C, :]
    w2b = np.concatenate([f(inputs["W2b"]).T, f(inputs["b2b"])[None, :]], axis=0)
    mb[0:H2 + 1, B_W2B:B_W2B + NO] = w2b

    mc = np.zeros((15, C_F), np.float32)
    mc[:, 0:H1] = np.concatenate(
        [f(inputs["W1"]).T, f(inputs["b1"])[None, :]], axis=0)
    mc[14, H1] = 1.0                 # unit col -> l1 slot computes to 1.0
    x_ext = np.concatenate([
        f(inputs["state_inno"]), f(inputs["obs_inno"]),
        f(inputs["diff_state"]), f(inputs["diff_obs"]), [np.float32(1.0)],
    ])
    mc[:, H1 + 1] = x_ext
    mc[:, H1 + 2] = x_ext
    return {"mega_a": ma,
            "mega_b1": np.ascontiguousarray(mb[:, 0:B_WIH1]),
            "mega_b2": np.ascontiguousarray(mb[:, B_WIH1:B_WHH1]),
            "mega_b3": np.ascontiguousarray(mb[:, B_WHH1:B_F]),
            "mega_c": mc}


def kernel(**inputs):
    nc = _get_nc()
    in_map = pack_inputs(inputs)
    res = bass_utils.run_bass_kernel_spmd(nc, [in_map], core_ids=[0], trace=TRACE)
    kernel.last_result = res
    return np.asarray(res.results[0]["out"], np.float32).reshape(X_DIM, Y_DIM)


# revision 26
# speedup vs baseline: 1.9299x; 1.0420x over previous
"""KalmanNet SLAM DNN forward pass on a single Trainium2 NeuronCore.

Network: x(14) -> Linear(560)+ReLU -> GRUCell(145) -> GRUCell(145)
         -> Linear(40)+ReLU -> Linear(10) -> reshape (5,2)

~1.8MB of fp32 weights, single sample => memory-bound; replicate on one
core (per sharding hint).

Matvecs run weights-stationary on the TensorEngine in fp32r (single-pass
fp32; ~1e-4-class relative error, far inside the scale-relative gate).
fp32r requires an even moving free dim, so every activation vector is
kept in duplicated column pairs ([K,2] rhs -> [M,2] psum) end to end.

Host-side numpy packs everything into three partition-major DRAM images
(per-partition contiguous runs of 2-14KB => near-line-rate SWDGE
descriptors, ~5% padding total), weights pre-transposed to [K, M]
layout, biases folded as an extra weight row against a constant-1.0
input element, GRU gates padded 145->146 so output chunks are uniform
73 partitions, and the z-gate pad-column bias set to 100 so the h'
garbage slot computes to exactly the 1.0 the next bias row needs.

Pointwise GRU math on VectorE, Sigmoid/Tanh on ScalarE (table-set load
pulled to t=0 by a dummy op), plus a dummy-matmul burst to warm the PE
clock during the DMA window.
"""

import numpy as np

import concourse.bacc as bacc
import concourse.mybir as mybir
import concourse.tile as tile
from concourse import bass_utils

F32 = mybir.dt.float32
F32R = mybir.dt.float32r
AF = mybir.ActivationFunctionType

X_DIM, Y_DIM = 5, 2
H1, H2 = 560, 40
G = 145          # GRU hidden size
C = 73           # partition chunk for the GRU state (2*73 = 146 = G+1)
GP = 2 * C       # per-gate padded column block
M3 = 3 * GP      # 438 padded gate columns
NO = X_DIM * Y_DIM

# megaB (128-partition image) column map
B_H0, B_H1 = 0, 4                      # h ptwise pairs [73,4]: (c0,c0,c1,c1)
B_HK = 8                               # whh rhs chunks: h0[128]x2, h1[128]x2,
                                       # tails (h0 rows 64:82, h1 rows 96:114)
B_WHH0 = 14                            # [128, 438] K-chunk 0
B_WHH1 = B_WHH0 + M3                   # [128, 438] K-chunk 0
B_TAILS = B_WHH1 + M3                  # rows 0:49 wih0c4, 64:82 whh0t, 96:114 whh1t
B_WIH1 = B_TAILS + M3                  # 2 x 438 (73-row chunks)
B_W2A = B_WIH1 + 2 * M3                # 2 x 41 (41st col makes the 1.0)
B_W2B = B_W2A + 2 * (H2 + 1)           # [41,10]
B_F = B_W2B + NO                       # 2296

A_F = 4 * M3                           # mega128: wih0 chunks c0..c3
C_F = H1 + 3                           # megaC: W1T(561, unit col) + x pair

TRACE = False
_BUILT = None


def _emit_gru(nc, pp, ab, name, wih_chunks, whh_chunks, h_sb,
              ptag_rz, ptag_ni, ptag_nh, hh_first=False):
    """One GRU cell, everything in duplicated column pairs.
    *_chunks: (lhsT[K, 438], rhs[K, 2]); h_sb: [73,4] prev hidden pairs
    with slots (72, 2:4) = 1.0. Returns h' [73,4] F32R pairs."""
    ps_rz = pp.tile([C, 8], F32, tag=ptag_rz)   # (r_c0, r_c1, z_c0, z_c1) pairs
    ps_ni = pp.tile([C, 4], F32, tag=ptag_ni)
    ps_nh = pp.tile([C, 4], F32, tag=ptag_nh)

    # One psum tile at a time (single accumulation bank phase each), r/z
    # first so the sigmoid overlaps the n-gate matmuls.
    def tp(chunk):
        return chunk[2] if len(chunk) > 2 else None

    rz_n = 4 * (len(wih_chunks) + len(whh_chunks))
    rz_i = 0
    rz_order = (whh_chunks + wih_chunks) if hh_first else (wih_chunks + whh_chunks)
    for ch in rz_order:
        lhsT, rhs = ch[0], ch[1]
        for g in (0, 1):
            for c in (0, 1):
                j = 2 * g + c
                nc.tensor.matmul(
                    ps_rz[:, 2 * j: 2 * j + 2],
                    lhsT[:, g * GP + C * c: g * GP + C * (c + 1)],
                    rhs, start=(rz_i == 0), stop=(rz_i == rz_n - 1),
                    tile_position=tp(ch))
                rz_i += 1
    for chunks, ps in ((whh_chunks, ps_nh), (wih_chunks, ps_ni)):
        n_i = 0
        for ch in chunks:
            lhsT, rhs = ch[0], ch[1]
            for c in (0, 1):
                nc.tensor.matmul(
                    ps[:, 2 * c: 2 * c + 2],
                    lhsT[:, 2 * GP + C * c: 2 * GP + C * (c + 1)],
                    rhs, start=(n_i == 0), stop=(n_i == 2 * len(chunks) - 1),
                    tile_position=tp(ch))
                n_i += 1

    # r,z = sigmoid(rz sums); n = tanh(i_n + r*h_n); h' = n + z*(h-n)
    rz = ab.tile([C, 8], F32, tag=f"{name}_rz")
    nc.scalar.activation(rz, ps_rz, AF.Sigmoid)
    t1 = ab.tile([C, 4], F32, tag=f"{name}_t1")
    nc.vector.tensor_mul(t1, rz[:, 0:4], ps_nh)
    nc.vector.tensor_add(t1, t1, ps_ni)
    n_sb = ab.tile([C, 4], F32, tag=f"{name}_n")
    nc.scalar.activation(n_sb, t1, AF.Tanh)
    d = ab.tile([C, 4], F32, tag=f"{name}_d")
    nc.vector.tensor_sub(d, h_sb.bitcast(F32), n_sb)
    nc.vector.tensor_mul(d, d, rz[:, 4:8])
    hp = ab.tile([C, 4], F32R, tag=f"{name}_hp")
    nc.vector.tensor_add(hp, n_sb, d)
    return hp


def _build():
    nc = bacc.Bacc("TRN2", num_devices=1, num_swdge_queues=4)

    d_a = nc.dram_tensor("mega_a", [128, A_F], F32R, kind="ExternalInput").ap()
    d_b1 = nc.dram_tensor("mega_b1", [128, B_WIH1], F32R, kind="ExternalInput").ap()
    d_b2 = nc.dram_tensor("mega_b2", [128, B_F - B_WIH1], F32R, kind="ExternalInput").ap()
    d_c = nc.dram_tensor("mega_c", [15, C_F], F32R, kind="ExternalInput").ap()
    d_out = nc.dram_tensor("out", [1, NO], F32, kind="ExternalOutput").ap()

    with tile.TileContext(nc) as tc:
        with (
            tc.tile_pool(name="wp", bufs=1) as wp,
            tc.tile_pool(name="ab", bufs=1) as ab,
            tc.tile_pool(name="pp", bufs=1, space="PSUM") as pp,
        ):
            # ACT table warmup
            warm = ab.tile([1, 1], F32, tag="warm")
            nc.vector.memset(warm, 0.0)
            warm2 = ab.tile([1, 1], F32, tag="warm2")
            nc.scalar.activation(warm2, warm, AF.Sigmoid)
            nc.scalar.activation(warm2, warm2, AF.Tanh)

            # --- DMAs (SWDGE), priority order ---
            mc = wp.tile([15, C_F], F32R, tag="mc")
            nc.sync.dma_start(mc, d_c)        # tiny; separate HWDGE ring
            ma = wp.tile([128, A_F], F32R, tag="ma")
            nc.gpsimd.dma_start(ma, d_a)
            mb = wp.tile([128, B_F], F32R, tag="mb")
            nc.gpsimd.dma_start(mb[:, 0:B_WIH1], d_b1)
            nc.gpsimd.dma_start(mb[:, B_WIH1:B_F], d_b2)

            # --- PE warmup: dummy fp32 matmuls (~3.5us of HAM activity) ---
            wz = ab.tile([128, 128], F32, tag="wz")
            nc.vector.memset(wz, 0.0)
            ps_w = pp.tile([2, 128], F32, tag="pw")
            for i in range(8):
                nc.tensor.matmul(ps_w, wz[:, 0:2], wz,
                                 start=(i == 0), stop=(i == 7))

            # --- layer 1: l1 = relu(W1 @ x + b1), [128,10] paired cols ---
            x2 = mc[:, H1 + 1:H1 + 3]
            ps_l1 = pp.tile([128, 8], F32, tag="p0")
            for c in range(4):
                nc.tensor.matmul(ps_l1[:, 2 * c:2 * c + 2],
                                 mc[:, c * 128:(c + 1) * 128], x2,
                                 start=(c == 0), stop=(c == 3))
            ps_l1b = pp.tile([49, 2], F32, tag="p5")
            nc.tensor.matmul(ps_l1b, mc[:, 512:561], x2, start=True, stop=True)
            l1_sb = ab.tile([128, 10], F32R, tag="l1")
            nc.vector.tensor_scalar_max(l1_sb[:, 0:8], ps_l1, 0.0)
            nc.vector.tensor_scalar_max(l1_sb[0:49, 8:10], ps_l1b, 0.0)

            h0_sb = mb[0:C, B_H0:B_H0 + 4]
            h1_sb = mb[0:C, B_H1:B_H1 + 4]

            # --- GRU 0 ---
            wih0_chunks = [
                (ma[:, c * M3:(c + 1) * M3], l1_sb[:, 2 * c:2 * c + 2])
                for c in range(4)
            ] + [
                (mb[0:49, B_TAILS:B_TAILS + M3], l1_sb[0:49, 8:10])
            ]
            whh0_chunks = [
                (mb[0:128, B_WHH0:B_WHH0 + M3], mb[0:128, B_HK:B_HK + 2]),
                (mb[64:82, B_TAILS:B_TAILS + M3], mb[64:82, B_HK + 4:B_HK + 6]),
            ]
            hp0 = _emit_gru(nc, pp, ab, "g0", wih0_chunks, whh0_chunks, h0_sb,
                            "p1", "p2", "p3")

            # --- GRU 1 ---
            wih1_chunks = [
                (mb[0:C, B_WIH1 + c * M3: B_WIH1 + (c + 1) * M3],
                 hp0[:, 2 * c:2 * c + 2])
                for c in range(2)
            ]
            whh1_chunks = [
                (mb[0:128, B_WHH1:B_WHH1 + M3], mb[0:128, B_HK + 2:B_HK + 4]),
                (mb[96:114, B_TAILS:B_TAILS + M3], mb[96:114, B_HK + 4:B_HK + 6], (96, 0)),
            ]
            hp1 = _emit_gru(nc, pp, ab, "g1", wih1_chunks, whh1_chunks, h1_sb,
                            "p0", "p1", "p2", hh_first=True)

            # --- l2 ---
            ps_a = pp.tile([H2 + 1, 2], F32, tag="p3")
            for c in range(2):
                nc.tensor.matmul(
                    ps_a, mb[0:C, B_W2A + c * (H2 + 1): B_W2A + (c + 1) * (H2 + 1)],
                    hp1[:, 2 * c:2 * c + 2], start=(c == 0), stop=(c == 1))
            l2h = ab.tile([H2 + 1, 2], F32R, tag="l2h")
            nc.vector.tensor_scalar_max(l2h, ps_a, 0.0)
            ps_o = pp.tile([1, NO], F32, tag="p4")
            nc.tensor.matmul(ps_o, l2h[:, 0:1],
                             mb[0:H2 + 1, B_W2B:B_W2B + NO],
                             start=True, stop=True)
            out_sb = ab.tile([1, NO], F32, tag="out_sb")
            nc.vector.tensor_copy(out_sb, ps_o)
            nc.sync.dma_start(d_out, out_sb)

    nc.compile()
    return nc


def _get_nc():
    global _BUILT
    if _BUILT is None:
        _BUILT = _build()
    return _BUILT


def _gate_pack(W, b, z_pad_bias=0.0):
    """W:(435,K), b:(435,) -> (K+1, 438): W.T + bias row, per-gate 146-col
    blocks (zero pad col). z_pad_bias=100 on the ih matrix makes the h'
    garbage slot compute to exactly 1.0."""
    K = W.shape[1]
    full = np.concatenate([W.T, b[None, :]], axis=0).astype(np.float32)
    out = np.zeros((K + 1, M3), np.float32)
    for g in range(3):
        out[:, g * GP: g * GP + G] = full[:, g * G: (g + 1) * G]
    out[K, GP + G] = z_pad_bias
    return out


def pack_inputs(inputs):
    f = lambda a: np.asarray(a, np.float32)
    wih0 = _gate_pack(f(inputs["Wih0"]), f(inputs["bih0"]), 100.0)  # (561, 438)
    ma = np.zeros((128, A_F), np.float32)
    for c in range(4):
        ma[:, c * M3:(c + 1) * M3] = wih0[c * 128:(c + 1) * 128, :]

    mb = np.zeros((128, B_F), np.float32)
    hn = f(inputs["hn"])
    for i, (col, h) in enumerate(((B_H0, hn[0]), (B_H1, hn[1]))):
        hx = np.append(h, np.float32(1.0))                 # (146,)
        v = hx.reshape(2, C).T                             # [73,2]
        mb[0:C, col:col + 4] = v[:, [0, 0, 1, 1]]          # ptwise pairs
        mb[0:128, B_HK + 2 * i:B_HK + 2 * i + 2] = hx[0:128, None]  # K-chunk rhs
        r0 = 64 if i == 0 else 96
        mb[r0:r0 + 18, B_HK + 4:B_HK + 6] = hx[128:146, None]
    mb[0:49, B_TAILS:B_TAILS + M3] = wih0[512:561, :]
    whh0 = _gate_pack(f(inputs["Whh0"]), f(inputs["bhh0"]))
    mb[0:128, B_WHH0:B_WHH0 + M3] = whh0[0:128, :]
    mb[64:82, B_TAILS:B_TAILS + M3] = whh0[128:146, :]
    whh1 = _gate_pack(f(inputs["Whh1"]), f(inputs["bhh1"]))
    mb[0:128, B_WHH1:B_WHH1 + M3] = whh1[0:128, :]
    mb[96:114, B_TAILS:B_TAILS + M3] = whh1[128:146, :]
    wih1 = _gate_pack(f(inputs["Wih1"]), f(inputs["bih1"]), 100.0)
    mb[0:C, B_WIH1:B_WIH1 + M3] = wih1[0:C, :]
    mb[0:C, B_WIH1 + M3:B_WIH1 + 2 * M3] = wih1[C:2 * C, :]
    w2a = np.zeros((2 * C, H2 + 1), np.float32)
    w2a[0:G + 1, 0:H2] = np.concatenate(
        [f(inputs["W2a"]).T, f(inputs["b2a"])[None, :]], axis=0)
    w2a[G, H2] = 1.0                 # unit col -> l2h slot computes to 1.0
    mb[0:C, B_W2A:B_W2A + H2 + 1] = w2a[0:C, :]
    mb[0:C, B_W2A + H2 + 1:B_W2A + 2 * (H2 + 1)] = w2a[C:2 * C, :]
    w2b = np.concatenate([f(inputs["W2b"]).T, f(inputs["b2b"])[None, :]], axis=0)
    mb[0:H2 + 1, B_W2B:B_W2B + NO] = w2b

    mc = np.zeros((15, C_F), np.float32)
    mc[:, 0:H1] = np.concatenate(
        [f(inputs["W1"]).T, f(inputs["b1"])[None, :]], axis=0)
    mc[14, H1] = 1.0                 # unit col -> l1 slot computes to 1.0
    x_ext = np.concatenate([
        f(inputs["state_inno"]), f(inputs["obs_inno"]),
        f(inputs["diff_state"]), f(inputs["diff_obs"]), [np.float32(1.0)],
    ])
    mc[:, H1 + 1] = x_ext
    mc[:, H1 + 2] = x_ext
    return {"mega_a": ma,
            "mega_b1": np.ascontiguousarray(mb[:, 0:B_WIH1]),
            "mega_b2": np.ascontiguousarray(mb[:, B_WIH1:B_F]),
            "mega_c": mc}


def kernel(**inputs):
    nc = _get_nc()
    in_map = pack_inputs(inputs)
    res = bass_utils.run_bass_kernel_spmd(nc, [in_map], core_ids=[0], trace=TRACE)
    kernel.last_result = res
    return np.asarray(res.results[0]["out"], np.float32).reshape(X_DIM, Y_DIM)
